# revision 3
# baseline (speedup 1.0000x reference)
"""Expert-parallel DeepseekV2 MoE kernel for 8 Trainium2 NeuronCores.

Strategy (v3):
  - Host computes the gate routing in numpy (mirrors the reference's grouped
    top-k exactly), gathers each expert's assigned tokens, and packs them
    into ragged per-position slots; the device runs the expert FFNs and the
    tensor-parallel shared MLP; host applies the combine scatter-add.
  - All weight/activation streams are bf16 (halves HBM traffic vs fp32;
    PSUM accumulation stays fp32; l2 rel err ~4e-3, well under the 2e-2
    gate). Matmuls run at 1 cycle/row either way, so bf16 is pure DMA win.
  - Ragged slot capacities: experts are split into near-equal pieces of at
    most 256 tokens; pieces are sorted by size and grouped into NSLOT
    positions of 8 (one piece per core). Each position's token count C_j is
    a compile-time constant = the max piece size in that position, so the
    matmul moving dims match the actual token counts instead of a fixed 256.
  - Gathered activations are laid out partition-major on host ([128, KT, TOT])
    so the device DMA is a plain contiguous slice copy.
  - Program cache is keyed by the capacity vector (fixed for a fixed routing).
"""

import numpy as np

import concourse.bass as bass
import concourse.tile as tile
from concourse import bacc, mybir
from concourse.bass_utils import run_bass_kernel_spmd

# Problem shapes (hardcoded per the harness contract).
T, D = 1024, 2048
E, I = 32, 1408
TOPK = 6
N_GROUP, TOPK_GROUP = 8, 3
ROUTED_SCALE = 2.5
SHARED_I = 2 * I  # 2816

NCORES = 8
ISH = SHARED_I // NCORES   # 352 shared-intermediate per core
CAPMAX = 256               # max tokens per piece (PSUM bank packing limit)
KT = D // 128              # 16 contraction tiles over D
IT = I // 128              # 11 intermediate tiles
DCH = D // 512             # 4 output chunks of 512
IS_SZ = [128, 128, ISH - 256]   # shared-intermediate tile sizes [128,128,96]

F32 = mybir.dt.float32
BF16 = mybir.dt.bfloat16
NP_BF16 = mybir.dt.np(mybir.dt.bfloat16)
SILU = mybir.ActivationFunctionType.Silu

_PROGRAM_CACHE = {}


def _build_program(caps):
    """caps: tuple of per-position token capacities C_j (each 1..256)."""
    nslot = len(caps)
    cpads = [-(-c // 128) * 128 for c in caps]
    offs = np.concatenate([[0], np.cumsum(cpads)]).astype(int)
    tot = int(offs[-1])

    nc = bacc.Bacc("TRN2", target_bir_lowering=False, debug=False)

    xg = nc.dram_tensor("xg", [128, KT, tot], BF16, kind="ExternalInput").ap()
    wg = nc.dram_tensor("wg", [nslot, D, I], BF16, kind="ExternalInput").ap()
    wu = nc.dram_tensor("wu", [nslot, D, I], BF16, kind="ExternalInput").ap()
    wd = nc.dram_tensor("wd", [nslot, I, D], BF16, kind="ExternalInput").ap()
    xt = nc.dram_tensor("xt", [D, T], BF16, kind="ExternalInput").ap()
    wsg = nc.dram_tensor("wsg", [D, ISH], BF16, kind="ExternalInput").ap()
    wsu = nc.dram_tensor("wsu", [D, ISH], BF16, kind="ExternalInput").ap()
    wsd = nc.dram_tensor("wsd", [ISH, D], BF16, kind="ExternalInput").ap()
    ye = nc.dram_tensor("ye", [tot, D], BF16, kind="ExternalOutput").ap()
    ys = nc.dram_tensor("ys", [T, D], BF16, kind="ExternalOutput").ap()

    with tile.TileContext(nc) as tc, \
         tc.tile_pool(name="psum", bufs=8, space="PSUM") as psum, \
         tc.tile_pool(name="shared_res", bufs=1) as shres, \
         tc.tile_pool(name="shared_tmp", bufs=1) as shtmp, \
         tc.tile_pool(name="ys_out", bufs=2) as yspool, \
         tc.tile_pool(name="xg_pool", bufs=2) as xgpool, \
         tc.tile_pool(name="wstream", bufs=3) as wpool, \
         tc.tile_pool(name="wdstream", bufs=3) as wdpool, \
         tc.tile_pool(name="hbufs", bufs=1) as hpool, \
         tc.tile_pool(name="ye_out", bufs=2) as yepool:
        # ---------------- shared-expert phase ----------------
        x_sb = shres.tile([128, KT, T], BF16, tag="x_sb")
        for h in range(2):
            nc.sync.dma_start(
                out=x_sb[:, h * 8:(h + 1) * 8, :],
                in_=xt[h * 1024:(h + 1) * 1024, :].rearrange(
                    "(a p) t -> p a t", p=128),
            )
        wsg_sb = shres.tile([128, KT, ISH], BF16, tag="wsg_sb")
        nc.sync.dma_start(out=wsg_sb[:], in_=wsg.rearrange("(a p) i -> p a i", p=128))
        wsu_sb = shres.tile([128, KT, ISH], BF16, tag="wsu_sb")
        nc.sync.dma_start(out=wsu_sb[:], in_=wsu.rearrange("(a p) i -> p a i", p=128))
        wsd_sb = shres.tile([128, 3, D], BF16, tag="wsd_sb")
        for j in range(3):
            sz = IS_SZ[j]
            nc.sync.dma_start(out=wsd_sb[:sz, j, :],
                              in_=wsd[j * 128:j * 128 + sz, :])

        # m1: hs^T[i_s, t] = silu(wsg^T x) * (wsu^T x), tiles [<=128, 512]
        hsg_sb = shtmp.tile([128, 3, T], F32, tag="hsg")
        hs_sb = shtmp.tile([128, 3, T], BF16, tag="hs")
        for w_sb, is_gate in ((wsg_sb, True), (wsu_sb, False)):
            ps = [psum.tile([128, 512], F32, tag="ps", name=f"ps{_i}")
                  for _i in range(6)]
            for k in range(KT):
                for j in range(3):
                    sz = IS_SZ[j]
                    for tch in range(2):
                        nc.tensor.matmul(
                            ps[j * 2 + tch][:sz, :],
                            w_sb[:, k, j * 128:j * 128 + sz],
                            x_sb[:, k, tch * 512:(tch + 1) * 512],
                            start=(k == 0), stop=(k == KT - 1),
                        )
            for j in range(3):
                sz = IS_SZ[j]
                for tch in range(2):
                    tsl = slice(tch * 512, (tch + 1) * 512)
                    if is_gate:
                        nc.scalar.activation(hsg_sb[:sz, j, tsl],
                                             ps[j * 2 + tch][:sz, :], SILU)
                    else:
                        nc.vector.tensor_mul(hs_sb[:sz, j, tsl],
                                             ps[j * 2 + tch][:sz, :],
                                             hsg_sb[:sz, j, tsl])

        # m2: ys[t, d] = hs^T.T @ wsd   (stationary hs^T, moving wsd)
        for tt in range(T // 128):
            ysb = yspool.tile([128, D], BF16, tag="ysb")
            for dc in range(DCH):
                py = psum.tile([128, 512], F32, tag="ps", name="pym2")
                for j in range(3):
                    sz = IS_SZ[j]
                    nc.tensor.matmul(
                        py[:],
                        hs_sb[:sz, j, tt * 128:(tt + 1) * 128],
                        wsd_sb[:sz, j, dc * 512:(dc + 1) * 512],
                        start=(j == 0), stop=(j == 2),
                    )
                nc.vector.tensor_copy(ysb[:, dc * 512:(dc + 1) * 512], py[:])
            nc.scalar.dma_start(out=ys[tt * 128:(tt + 1) * 128, :], in_=ysb[:])

        # ---------------- routed-expert phase ----------------
        for e in range(nslot):
            C = caps[e]
            CP = cpads[e]
            TTE = -(-C // 128)          # token tiles in this slot
            off = int(offs[e])
            xg_sb = xgpool.tile([128, KT, CAPMAX], BF16, tag="xg_sb")
            nc.sync.dma_start(out=xg_sb[:, :, :CP], in_=xg[:, :, off:off + CP])

            hg_sb = hpool.tile([128, IT, CAPMAX], F32, tag="hg")
            h_sb = hpool.tile([128, IT, CAPMAX], BF16, tag="h")
            # gate then up: h^T tiles [128(i), C], stationary weights.
            # PSUM banks hold two i-tiles at 256-float offsets.
            for w_dram, is_gate in ((wg, True), (wu, False)):
                ps = [psum.tile([128, 512], F32, tag="ps", name=f"ps{_i}")
                      for _i in range(6)]
                for kc in range(KT // 4):
                    w_sb = wpool.tile([128, 4, I], BF16, tag="wst")
                    nc.sync.dma_start(
                        out=w_sb[:],
                        in_=w_dram[e, kc * 512:(kc + 1) * 512, :].rearrange(
                            "(a p) i -> p a i", p=128),
                    )
                    for a in range(4):
                        k = kc * 4 + a
                        for it in range(IT):
                            csl = slice((it % 2) * 256, (it % 2) * 256 + C)
                            last_in_pair = (it == IT - 1) or (it % 2 == 1)
                            nc.tensor.matmul(
                                ps[it // 2][:, csl],
                                w_sb[:, a, it * 128:(it + 1) * 128],
                                xg_sb[:, k, :C],
                                start=(k == 0 and it % 2 == 0),
                                stop=(k == KT - 1 and last_in_pair),
                            )
                for it in range(IT):
                    src = ps[it // 2][:, (it % 2) * 256:(it % 2) * 256 + C]
                    if is_gate:
                        nc.scalar.activation(hg_sb[:, it, :C], src, SILU)
                    else:
                        nc.vector.tensor_mul(h_sb[:, it, :C], src,
                                             hg_sb[:, it, :C])

            # down: y[t, d] = h^T.T @ wd  (stationary h^T, moving wd)
            ye_sb = yepool.tile([128, 2, D], BF16, tag="ye_sb")
            pys = [psum.tile([128, 512], F32, tag="ps", name=f"py{_i}")
                   for _i in range(4 * TTE)]
            ichunks = [(0, 2), (2, 2), (4, 2), (6, 2), (8, 2), (10, 1)]
            for i0, cnt in ichunks:
                wd_sb = wdpool.tile([128, 2, D], BF16, tag="wdst")
                nc.sync.dma_start(
                    out=wd_sb[:, :cnt, :],
                    in_=wd[e, i0 * 128:(i0 + cnt) * 128, :].rearrange(
                        "(a p) d -> p a d", p=128),
                )
                for a in range(cnt):
                    i = i0 + a
                    for tt in range(TTE):
                        tsz = min(128, C - tt * 128)
                        for dc in range(DCH):
                            nc.tensor.matmul(
                                pys[tt * DCH + dc][:tsz, :],
                                h_sb[:, i, tt * 128:tt * 128 + tsz],
                                wd_sb[:, a, dc * 512:(dc + 1) * 512],
                                start=(i == 0), stop=(i == IT - 1),
                            )
            for tt in range(TTE):
                for dc in range(DCH):
                    nc.vector.tensor_copy(ye_sb[:, tt, dc * 512:(dc + 1) * 512],
                                          pys[tt * DCH + dc][:])
            nc.scalar.dma_start(
                out=ye[off:off + CP].rearrange("(a p) d -> p a d", p=128),
                in_=ye_sb[:, :TTE, :])

    nc.compile()
    return nc


def get_program(caps):
    key = tuple(caps)
    if key not in _PROGRAM_CACHE:
        _PROGRAM_CACHE[key] = _build_program(key)
    return _PROGRAM_CACHE[key]


def _route_numpy(x, gate_w, bias):
    """Mirror reference.py's grouped top-k routing in fp32 numpy."""
    logits = x @ gate_w                                   # [T, E]
    scores = 1.0 / (1.0 + np.exp(-logits))
    sc = scores + bias[None, :]
    g = sc.reshape(-1, N_GROUP, E // N_GROUP)
    group_scores = np.sort(g, axis=-1)[..., -2:].sum(-1)  # [T, n_group]
    gidx = np.argsort(-group_scores, axis=-1, kind="stable")[:, :TOPK_GROUP]
    gmask = np.zeros((x.shape[0], N_GROUP), np.bool_)
    np.put_along_axis(gmask, gidx, True, axis=-1)
    emask = np.repeat(gmask, E // N_GROUP, axis=-1)       # [T, E]
    masked = np.where(emask, sc, -np.inf)
    topk_idx = np.argsort(-masked, axis=-1, kind="stable")[:, :TOPK]
    w = np.take_along_axis(scores, topk_idx, axis=-1)
    w = w / (w.sum(-1, keepdims=True) + 1e-20)
    return topk_idx, w


def _plan(topk_idx, topk_w):
    """Split experts into near-equal pieces (<= CAPMAX tokens), sort pieces by
    size, group into positions of NCORES; position capacity = max piece size.

    Returns per_core[c] = list of (expert, token_idx, weights) per position,
    and caps = tuple of position capacities.
    """
    flat_e = topk_idx.ravel()
    flat_t = np.repeat(np.arange(T), TOPK)
    flat_w = (topk_w * ROUTED_SCALE).ravel().astype(np.float32)
    order = np.argsort(flat_e, kind="stable")
    sorted_t = flat_t[order]
    sorted_w = flat_w[order]
    counts = np.bincount(flat_e, minlength=E)
    offsets = np.concatenate([[0], np.cumsum(counts)])

    pieces = []  # (expert, token_idx, weights)
    for e in range(E):
        toks = sorted_t[offsets[e]:offsets[e + 1]]
        ws = sorted_w[offsets[e]:offsets[e + 1]]
        n = len(toks)
        k = max(1, -(-n // CAPMAX))
        bounds = np.linspace(0, n, k + 1).astype(int)
        for b in range(k):
            pieces.append((e, toks[bounds[b]:bounds[b + 1]],
                           ws[bounds[b]:bounds[b + 1]]))
    pieces.sort(key=lambda p: -len(p[1]))
    nslot = -(-len(pieces) // NCORES)
    while len(pieces) < nslot * NCORES:
        pieces.append((0, np.empty(0, np.int64), np.empty(0, np.float32)))

    caps = []
    per_core = [[] for _ in range(NCORES)]
    for j in range(nslot):
        grp = pieces[j * NCORES:(j + 1) * NCORES]
        caps.append(max(1, max(len(p[1]) for p in grp)))
        for c in range(NCORES):
            per_core[c].append(grp[c])
    return per_core, tuple(caps)


def build_in_maps(inputs):
    """Route, pack pieces, and build the per-core device input maps."""
    x = np.ascontiguousarray(np.asarray(inputs["hidden_states"], np.float32))
    gate_w = np.asarray(inputs["gate_w"], np.float32)
    bias = np.asarray(inputs["e_score_correction_bias"], np.float32)
    w_gate = np.asarray(inputs["w_gate"], np.float32).astype(NP_BF16)
    w_up = np.asarray(inputs["w_up"], np.float32).astype(NP_BF16)
    w_down = np.asarray(inputs["w_down"], np.float32).astype(NP_BF16)
    ws_gate = np.asarray(inputs["ws_gate"], np.float32).astype(NP_BF16)
    ws_up = np.asarray(inputs["ws_up"], np.float32).astype(NP_BF16)
    ws_down = np.asarray(inputs["ws_down"], np.float32).astype(NP_BF16)

    topk_idx, topk_w = _route_numpy(x, gate_w, bias)
    per_core, caps = _plan(topk_idx, topk_w)
    nslot = len(caps)
    cpads = [-(-c // 128) * 128 for c in caps]
    offs = np.concatenate([[0], np.cumsum(cpads)]).astype(int)
    tot = int(offs[-1])

    x_t = np.ascontiguousarray(x.T.astype(NP_BF16))  # [D, T] bf16
    # partition-major x for gathering: [128, KT, T]
    x_pm = np.ascontiguousarray(
        x_t.reshape(KT, 128, T).transpose(1, 0, 2))

    in_maps = []
    for c in range(NCORES):
        xg_np = np.zeros((128, KT, tot), NP_BF16)
        wg_np = np.empty((nslot, D, I), NP_BF16)
        wu_np = np.empty((nslot, D, I), NP_BF16)
        wd_np = np.empty((nslot, I, D), NP_BF16)
        for j in range(nslot):
            e, idx, _ = per_core[c][j]
            if len(idx):
                off = int(offs[j])
                xg_np[:, :, off:off + len(idx)] = x_pm[:, :, idx]
            wg_np[j] = w_gate[e]
            wu_np[j] = w_up[e]
            wd_np[j] = w_down[e]
        in_maps.append({
            "xg": xg_np, "wg": wg_np, "wu": wu_np, "wd": wd_np,
            "xt": x_t,
            "wsg": np.ascontiguousarray(ws_gate[:, c * ISH:(c + 1) * ISH]),
            "wsu": np.ascontiguousarray(ws_up[:, c * ISH:(c + 1) * ISH]),
            "wsd": np.ascontiguousarray(ws_down[c * ISH:(c + 1) * ISH, :]),
        })
    return in_maps, per_core, caps


def kernel(**inputs):
    in_maps, per_core, caps = build_in_maps(inputs)
    nc = get_program(caps)
    res = run_bass_kernel_spmd(nc, in_maps, core_ids=list(range(NCORES)))

    cpads = [-(-c // 128) * 128 for c in caps]
    offs = np.concatenate([[0], np.cumsum(cpads)]).astype(int)

    routed = np.zeros((T, D), np.float32)
    shared = np.zeros((T, D), np.float32)
    for c in range(NCORES):
        ye = res.results[c]["ye"]
        for j, (e, idx, wv) in enumerate(per_core[c]):
            if not len(idx):
                continue
            off = int(offs[j])
            y = np.asarray(ye[off:off + len(idx)], np.float32)
            routed[idx] += wv[:, None] * y
        shared += np.asarray(res.results[c]["ys"], np.float32)

    return (routed + shared).astype(np.float32)


# revision 4
# speedup vs baseline: 1.0507x; 1.0507x over previous
"""Expert-parallel DeepseekV2 MoE kernel for 8 Trainium2 NeuronCores.

Design (v5):
  - All weight/activation streams in bf16 (halves HBM traffic; PSUM stays
    fp32; l2 rel err ~4e-3 vs the 2e-2 gate).
  - Shared-expert and routed-expert tile pools coexist so routed weight
    prefetch overlaps shared compute; initial x/shared-weight DMAs are
    chunked so the first matmul starts after ~1MB instead of ~3.5MB.
  - Ragged slot capacities: experts are split into near-equal pieces of at
    most 256 tokens; pieces are sorted by size and grouped into NSLOT
    positions of 8 (one piece per core). Each position's token count C_j is
    a compile-time constant = the max piece size in that position, so the
    matmul moving dims match the actual token counts instead of a fixed 256.
  - Gathered activations are laid out partition-major on host ([128, KT, TOT])
    so the device DMA is a plain contiguous slice copy.
  - Program cache is keyed by the capacity vector (fixed for a fixed routing).
"""

import numpy as np

import concourse.bass as bass
import concourse.tile as tile
from concourse import bacc, mybir
from concourse.bass_utils import run_bass_kernel_spmd

# Problem shapes (hardcoded per the harness contract).
T, D = 1024, 2048
E, I = 32, 1408
TOPK = 6
N_GROUP, TOPK_GROUP = 8, 3
ROUTED_SCALE = 2.5
SHARED_I = 2 * I  # 2816

NCORES = 8
ISH = SHARED_I // NCORES   # 352 shared-intermediate per core
CAPMAX = 256               # max tokens per piece (PSUM bank packing limit)
KT = D // 128              # 16 contraction tiles over D
IT = I // 128              # 11 intermediate tiles
DCH = D // 512             # 4 output chunks of 512
IS_SZ = [128, 128, ISH - 256]   # shared-intermediate tile sizes [128,128,96]

F32 = mybir.dt.float32
BF16 = mybir.dt.bfloat16
NP_BF16 = mybir.dt.np(mybir.dt.bfloat16)
SILU = mybir.ActivationFunctionType.Silu

_PROGRAM_CACHE = {}


def _build_program(caps):
    """caps: tuple of per-position token capacities C_j (each 1..256)."""
    nslot = len(caps)
    cpads = [-(-c // 128) * 128 for c in caps]
    offs = np.concatenate([[0], np.cumsum(cpads)]).astype(int)
    tot = int(offs[-1])

    nc = bacc.Bacc("TRN2", target_bir_lowering=False, debug=False)

    xg = nc.dram_tensor("xg", [128, KT, tot], BF16, kind="ExternalInput").ap()
    wg = nc.dram_tensor("wg", [nslot, D, I], BF16, kind="ExternalInput").ap()
    wu = nc.dram_tensor("wu", [nslot, D, I], BF16, kind="ExternalInput").ap()
    wd = nc.dram_tensor("wd", [nslot, I, D], BF16, kind="ExternalInput").ap()
    xt = nc.dram_tensor("xt", [D, T], BF16, kind="ExternalInput").ap()
    wsg = nc.dram_tensor("wsg", [D, ISH], BF16, kind="ExternalInput").ap()
    wsu = nc.dram_tensor("wsu", [D, ISH], BF16, kind="ExternalInput").ap()
    wsd = nc.dram_tensor("wsd", [ISH, D], BF16, kind="ExternalInput").ap()
    ye = nc.dram_tensor("ye", [tot, D], BF16, kind="ExternalOutput").ap()
    ys = nc.dram_tensor("ys", [T, D], BF16, kind="ExternalOutput").ap()

    with tile.TileContext(nc) as tc, \
         tc.tile_pool(name="psum", bufs=8, space="PSUM") as psum, \
         tc.tile_pool(name="shared_res", bufs=1) as shres, \
         tc.tile_pool(name="shared_tmp", bufs=1) as shtmp, \
         tc.tile_pool(name="ys_out", bufs=2) as yspool, \
         tc.tile_pool(name="xg_pool", bufs=2) as xgpool, \
         tc.tile_pool(name="wstream", bufs=4) as wpool, \
         tc.tile_pool(name="wdstream", bufs=3) as wdpool, \
         tc.tile_pool(name="hbufs", bufs=1) as hpool, \
         tc.tile_pool(name="ye_out", bufs=1) as yepool:
        # ---------------- shared-expert phase ----------------
        x_sb = shres.tile([128, KT, T], BF16, tag="x_sb")
        for h in range(4):
            nc.sync.dma_start(
                out=x_sb[:, h * 4:(h + 1) * 4, :],
                in_=xt[h * 512:(h + 1) * 512, :].rearrange(
                    "(a p) t -> p a t", p=128),
            )
        wsg_sb = shres.tile([128, KT, ISH], BF16, tag="wsg_sb")
        for h in range(2):
            nc.sync.dma_start(
                out=wsg_sb[:, h * 8:(h + 1) * 8, :],
                in_=wsg[h * 1024:(h + 1) * 1024, :].rearrange(
                    "(a p) i -> p a i", p=128))
        wsu_sb = shres.tile([128, KT, ISH], BF16, tag="wsu_sb")
        for h in range(2):
            nc.sync.dma_start(
                out=wsu_sb[:, h * 8:(h + 1) * 8, :],
                in_=wsu[h * 1024:(h + 1) * 1024, :].rearrange(
                    "(a p) i -> p a i", p=128))
        wsd_sb = shres.tile([128, 3, D], BF16, tag="wsd_sb")
        for j in range(3):
            sz = IS_SZ[j]
            nc.sync.dma_start(out=wsd_sb[:sz, j, :],
                              in_=wsd[j * 128:j * 128 + sz, :])

        # m1: hs^T[i_s, t] = silu(wsg^T x) * (wsu^T x), tiles [<=128, 512]
        hsg_sb = shtmp.tile([128, 3, T], F32, tag="hsg")
        hs_sb = shtmp.tile([128, 3, T], BF16, tag="hs")
        for w_sb, is_gate in ((wsg_sb, True), (wsu_sb, False)):
            ps = [psum.tile([128, 512], F32, tag="ps", name=f"ps{_i}")
                  for _i in range(6)]
            for k in range(KT):
                for j in range(3):
                    sz = IS_SZ[j]
                    for tch in range(2):
                        nc.tensor.matmul(
                            ps[j * 2 + tch][:sz, :],
                            w_sb[:, k, j * 128:j * 128 + sz],
                            x_sb[:, k, tch * 512:(tch + 1) * 512],
                            start=(k == 0), stop=(k == KT - 1),
                        )
            for j in range(3):
                sz = IS_SZ[j]
                for tch in range(2):
                    tsl = slice(tch * 512, (tch + 1) * 512)
                    if is_gate:
                        nc.scalar.activation(hsg_sb[:sz, j, tsl],
                                             ps[j * 2 + tch][:sz, :], SILU)
                    else:
                        nc.vector.tensor_mul(hs_sb[:sz, j, tsl],
                                             ps[j * 2 + tch][:sz, :],
                                             hsg_sb[:sz, j, tsl])

        # m2: ys[t, d] = hs^T.T @ wsd   (stationary hs^T, moving wsd)
        for tt in range(T // 128):
            ysb = yspool.tile([128, D], BF16, tag="ysb")
            for dc in range(DCH):
                py = psum.tile([128, 512], F32, tag="ps", name="pym2")
                for j in range(3):
                    sz = IS_SZ[j]
                    nc.tensor.matmul(
                        py[:],
                        hs_sb[:sz, j, tt * 128:(tt + 1) * 128],
                        wsd_sb[:sz, j, dc * 512:(dc + 1) * 512],
                        start=(j == 0), stop=(j == 2),
                    )
                nc.vector.tensor_copy(ysb[:, dc * 512:(dc + 1) * 512], py[:])
            nc.scalar.dma_start(out=ys[tt * 128:(tt + 1) * 128, :], in_=ysb[:])

        # ---------------- routed-expert phase ----------------
        for e in range(nslot):
            C = caps[e]
            CP = cpads[e]
            TTE = -(-C // 128)          # token tiles in this slot
            off = int(offs[e])
            xg_sb = xgpool.tile([128, KT, CAPMAX], BF16, tag="xg_sb")
            nc.sync.dma_start(out=xg_sb[:, :, :CP], in_=xg[:, :, off:off + CP])

            hg_sb = hpool.tile([128, IT, CAPMAX], F32, tag="hg")
            h_sb = hpool.tile([128, IT, CAPMAX], BF16, tag="h")
            # gate then up: h^T tiles [128(i), C], stationary weights.
            # PSUM banks hold two i-tiles at 256-float offsets.
            for w_dram, is_gate in ((wg, True), (wu, False)):
                ps = [psum.tile([128, 512], F32, tag="ps", name=f"ps{_i}")
                      for _i in range(6)]
                for kc in range(KT // 4):
                    w_sb = wpool.tile([128, 4, I], BF16, tag="wst")
                    nc.sync.dma_start(
                        out=w_sb[:],
                        in_=w_dram[e, kc * 512:(kc + 1) * 512, :].rearrange(
                            "(a p) i -> p a i", p=128),
                    )
                    for a in range(4):
                        k = kc * 4 + a
                        for it in range(IT):
                            csl = slice((it % 2) * 256, (it % 2) * 256 + C)
                            last_in_pair = (it == IT - 1) or (it % 2 == 1)
                            nc.tensor.matmul(
                                ps[it // 2][:, csl],
                                w_sb[:, a, it * 128:(it + 1) * 128],
                                xg_sb[:, k, :C],
                                start=(k == 0 and it % 2 == 0),
                                stop=(k == KT - 1 and last_in_pair),
                            )
                for it in range(IT):
                    src = ps[it // 2][:, (it % 2) * 256:(it % 2) * 256 + C]
                    if is_gate:
                        nc.scalar.activation(hg_sb[:, it, :C], src, SILU)
                    else:
                        nc.vector.tensor_mul(h_sb[:, it, :C], src,
                                             hg_sb[:, it, :C])

            # down: y[t, d] = h^T.T @ wd  (stationary h^T, moving wd)
            ye_sb = yepool.tile([128, 2, D], BF16, tag="ye_sb")
            pys = [psum.tile([128, 512], F32, tag="ps", name=f"py{_i}")
                   for _i in range(4 * TTE)]
            ichunks = [(0, 2), (2, 2), (4, 2), (6, 2), (8, 2), (10, 1)]
            for i0, cnt in ichunks:
                wd_sb = wdpool.tile([128, 2, D], BF16, tag="wdst")
                nc.sync.dma_start(
                    out=wd_sb[:, :cnt, :],
                    in_=wd[e, i0 * 128:(i0 + cnt) * 128, :].rearrange(
                        "(a p) d -> p a d", p=128),
                )
                for a in range(cnt):
                    i = i0 + a
                    for tt in range(TTE):
                        tsz = min(128, C - tt * 128)
                        for dc in range(DCH):
                            nc.tensor.matmul(
                                pys[tt * DCH + dc][:tsz, :],
                                h_sb[:, i, tt * 128:tt * 128 + tsz],
                                wd_sb[:, a, dc * 512:(dc + 1) * 512],
                                start=(i == 0), stop=(i == IT - 1),
                            )
            for tt in range(TTE):
                for dc in range(DCH):
                    nc.vector.tensor_copy(ye_sb[:, tt, dc * 512:(dc + 1) * 512],
                                          pys[tt * DCH + dc][:])
            nc.scalar.dma_start(
                out=ye[off:off + CP].rearrange("(a p) d -> p a d", p=128),
                in_=ye_sb[:, :TTE, :])

    nc.compile()
    return nc


def get_program(caps):
    key = tuple(caps)
    if key not in _PROGRAM_CACHE:
        _PROGRAM_CACHE[key] = _build_program(key)
    return _PROGRAM_CACHE[key]


def _route_numpy(x, gate_w, bias):
    """Mirror reference.py's grouped top-k routing in fp32 numpy."""
    logits = x @ gate_w                                   # [T, E]
    scores = 1.0 / (1.0 + np.exp(-logits))
    sc = scores + bias[None, :]
    g = sc.reshape(-1, N_GROUP, E // N_GROUP)
    group_scores = np.sort(g, axis=-1)[..., -2:].sum(-1)  # [T, n_group]
    gidx = np.argsort(-group_scores, axis=-1, kind="stable")[:, :TOPK_GROUP]
    gmask = np.zeros((x.shape[0], N_GROUP), np.bool_)
    np.put_along_axis(gmask, gidx, True, axis=-1)
    emask = np.repeat(gmask, E // N_GROUP, axis=-1)       # [T, E]
    masked = np.where(emask, sc, -np.inf)
    topk_idx = np.argsort(-masked, axis=-1, kind="stable")[:, :TOPK]
    w = np.take_along_axis(scores, topk_idx, axis=-1)
    w = w / (w.sum(-1, keepdims=True) + 1e-20)
    return topk_idx, w


def _plan(topk_idx, topk_w):
    """Split experts into near-equal pieces (<= CAPMAX tokens), sort pieces by
    size, group into positions of NCORES; position capacity = max piece size.

    Returns per_core[c] = list of (expert, token_idx, weights) per position,
    and caps = tuple of position capacities.
    """
    flat_e = topk_idx.ravel()
    flat_t = np.repeat(np.arange(T), TOPK)
    flat_w = (topk_w * ROUTED_SCALE).ravel().astype(np.float32)
    order = np.argsort(flat_e, kind="stable")
    sorted_t = flat_t[order]
    sorted_w = flat_w[order]
    counts = np.bincount(flat_e, minlength=E)
    offsets = np.concatenate([[0], np.cumsum(counts)])

    pieces = []  # (expert, token_idx, weights)
    for e in range(E):
        toks = sorted_t[offsets[e]:offsets[e + 1]]
        ws = sorted_w[offsets[e]:offsets[e + 1]]
        n = len(toks)
        k = max(1, -(-n // CAPMAX))
        bounds = np.linspace(0, n, k + 1).astype(int)
        for b in range(k):
            pieces.append((e, toks[bounds[b]:bounds[b + 1]],
                           ws[bounds[b]:bounds[b + 1]]))
    pieces.sort(key=lambda p: -len(p[1]))
    nslot = -(-len(pieces) // NCORES)
    while len(pieces) < nslot * NCORES:
        pieces.append((0, np.empty(0, np.int64), np.empty(0, np.float32)))

    caps = []
    per_core = [[] for _ in range(NCORES)]
    for j in range(nslot):
        grp = pieces[j * NCORES:(j + 1) * NCORES]
        caps.append(max(1, max(len(p[1]) for p in grp)))
        for c in range(NCORES):
            per_core[c].append(grp[c])
    return per_core, tuple(caps)


def build_in_maps(inputs):
    """Route, pack pieces, and build the per-core device input maps."""
    x = np.ascontiguousarray(np.asarray(inputs["hidden_states"], np.float32))
    gate_w = np.asarray(inputs["gate_w"], np.float32)
    bias = np.asarray(inputs["e_score_correction_bias"], np.float32)
    w_gate = np.asarray(inputs["w_gate"], np.float32).astype(NP_BF16)
    w_up = np.asarray(inputs["w_up"], np.float32).astype(NP_BF16)
    w_down = np.asarray(inputs["w_down"], np.float32).astype(NP_BF16)
    ws_gate = np.asarray(inputs["ws_gate"], np.float32).astype(NP_BF16)
    ws_up = np.asarray(inputs["ws_up"], np.float32).astype(NP_BF16)
    ws_down = np.asarray(inputs["ws_down"], np.float32).astype(NP_BF16)

    topk_idx, topk_w = _route_numpy(x, gate_w, bias)
    per_core, caps = _plan(topk_idx, topk_w)
    nslot = len(caps)
    cpads = [-(-c // 128) * 128 for c in caps]
    offs = np.concatenate([[0], np.cumsum(cpads)]).astype(int)
    tot = int(offs[-1])

    x_t = np.ascontiguousarray(x.T.astype(NP_BF16))  # [D, T] bf16
    # partition-major x for gathering: [128, KT, T]
    x_pm = np.ascontiguousarray(
        x_t.reshape(KT, 128, T).transpose(1, 0, 2))

    in_maps = []
    for c in range(NCORES):
        xg_np = np.zeros((128, KT, tot), NP_BF16)
        wg_np = np.empty((nslot, D, I), NP_BF16)
        wu_np = np.empty((nslot, D, I), NP_BF16)
        wd_np = np.empty((nslot, I, D), NP_BF16)
        for j in range(nslot):
            e, idx, _ = per_core[c][j]
            if len(idx):
                off = int(offs[j])
                xg_np[:, :, off:off + len(idx)] = x_pm[:, :, idx]
            wg_np[j] = w_gate[e]
            wu_np[j] = w_up[e]
            wd_np[j] = w_down[e]
        in_maps.append({
            "xg": xg_np, "wg": wg_np, "wu": wu_np, "wd": wd_np,
            "xt": x_t,
            "wsg": np.ascontiguousarray(ws_gate[:, c * ISH:(c + 1) * ISH]),
            "wsu": np.ascontiguousarray(ws_up[:, c * ISH:(c + 1) * ISH]),
            "wsd": np.ascontiguousarray(ws_down[c * ISH:(c + 1) * ISH, :]),
        })
    return in_maps, per_core, caps


def kernel(**inputs):
    in_maps, per_core, caps = build_in_maps(inputs)
    nc = get_program(caps)
    res = run_bass_kernel_spmd(nc, in_maps, core_ids=list(range(NCORES)))

    cpads = [-(-c // 128) * 128 for c in caps]
    offs = np.concatenate([[0], np.cumsum(cpads)]).astype(int)

    routed = np.zeros((T, D), np.float32)
    shared = np.zeros((T, D), np.float32)
    for c in range(NCORES):
        ye = res.results[c]["ye"]
        for j, (e, idx, wv) in enumerate(per_core[c]):
            if not len(idx):
                continue
            off = int(offs[j])
            y = np.asarray(ye[off:off + len(idx)], np.float32)
            routed[idx] += wv[:, None] * y
        shared += np.asarray(res.results[c]["ys"], np.float32)

    return (routed + shared).astype(np.float32)
